# revision 10
# baseline (speedup 1.0000x reference)
"""KNN WRMF negative sampler on 8 Trainium2 NeuronCores.

Data-parallel over L=4096 rows (512 rows/core, 4 tiles of 128 partitions).

Per row l with loc = trg_seq[l,1], for each of K=32 uniforms u:
  idx    = min(#{n: cum[loc,n] < u}, 99)
  neg    = knn[loc-1, idx]
  prob   = probs[loc, idx]        (approximated as cum[idx]-cum[idx-1])

Both take-alongs are evaluated as telescoped indicator sums so no
per-element gather is needed (TRN2 indirect DMA only supports one
offset per partition):

  A[n]  = 1[cum[n] < u]                       n = 0..99  (prefix mask)
  neg   = nb[0] + sum_n w_n[n] * A[n],  w_n[n] = nb[n+1]-nb[n]  (n<=98)
  prob  = c[0]  + sum_n w_p[n] * A[n],  w_p[n] = (c[n+1]-c[n])-(c[n]-c[n-1])

The clamp at 99 falls out of truncating the sums at n=98 (w[99]=0).
The weights, c0 and nb0 are precomputed on the host into one fused
2KB-row table so each tile needs a single [P,1]-offset indirect DMA.
neg arithmetic is exact (integer-valued f32 sums < 2^24); prob error is
~1e-5 relative.

Engine split per tile: DVE does GT + combined mul + prob-reduce;
gpsimd does the row gather and the knn-reduce.
"""

import numpy as np
from contextlib import ExitStack

import concourse.bass as bass
import concourse.bacc as bacc
import concourse.mybir as mybir
import concourse.tile as tile
from concourse.bass_utils import run_bass_kernel_spmd

P = 128          # partitions
T = 4            # row-tiles per core
RPC = P * T      # rows per core
K = 32           # samples per row
N = 100          # neighbours per row
FTW = 512        # fused table row width (f32 elems, 2KB)
NCORES = 8
NLOC = 100000

GPSIMD_REDUCE = True

_cache = {}


def _build():
    if "nc" in _cache:
        return _cache["nc"]
    nc = bacc.Bacc("TRN2")
    f32, i32, bf16 = mybir.dt.float32, mybir.dt.int32, mybir.dt.bfloat16
    trg = nc.dram_tensor("trg", [RPC, 2], i32, kind="ExternalInput").ap()
    uni = nc.dram_tensor("uni", [RPC, K], f32, kind="ExternalInput").ap()
    ftab = nc.dram_tensor("ftab", [NLOC + 1, FTW], f32, kind="ExternalInput").ap()
    oneg = nc.dram_tensor("oneg", [RPC, K], i32, kind="ExternalOutput").ap()
    oprob = nc.dram_tensor("oprob", [RPC, K], f32, kind="ExternalOutput").ap()

    GT = mybir.AluOpType.is_gt
    ADD = mybir.AluOpType.add
    MUL = mybir.AluOpType.mult
    X = mybir.AxisListType.X

    with tile.TileContext(nc) as tc, ExitStack() as ctx:
        pool = ctx.enter_context(tc.tile_pool(name="m", bufs=1))
        big = ctx.enter_context(tc.tile_pool(name="big", bufs=2))
        ftp = ctx.enter_context(tc.tile_pool(name="ftp", bufs=4))

        # row -> (tile t, partition p): l = t*128 + p
        loc = pool.tile([P, T], i32)
        nc.sync.dma_start(loc[:], trg[:, 1:2].rearrange("(t p) c -> p (t c)", p=P))
        ut = pool.tile([P, T, K], f32)
        nc.sync.dma_start(ut[:], uni.rearrange("(t p) k -> p t k", p=P))

        probs = pool.tile([P, T, K], f32)
        negf = pool.tile([P, T, K], f32)

        fts = []

        def gather(t):
            ft = ftp.tile([P, FTW], f32, tag="ft")
            nc.gpsimd.indirect_dma_start(
                out=ft[:], out_offset=None, in_=ftab[:],
                in_offset=bass.IndirectOffsetOnAxis(ap=loc[:, t:t + 1], axis=0))
            fts.append(ft)

        gather(0)
        for t in range(T):
            ft = fts[t]
            u_b = ut[:, t, :][:, :, None].to_broadcast([P, K, N])
            c_b = ft[:, 0:N][:, None, :].to_broadcast([P, K, N])
            A = big.tile([P, K, N], bf16, tag="A")
            nc.vector.tensor_tensor(out=A[:], in0=u_b, in1=c_b, op=GT)

            w2 = ft[:, N:3 * N].rearrange("p (s n) -> p s n", s=2)
            gp = GPSIMD_REDUCE and t + 1 < T
            if gp:
                # k-innermost product layout so the gpsimd add-tree reads
                # contiguous 4-byte-stride runs (avoids the Q7 fetch cliff)
                w_b = w2[:, :, :, None].to_broadcast([P, 2, N, K])
                a_b = (A[:].rearrange("p k n -> p n k")
                       [:, None, :, :].to_broadcast([P, 2, N, K]))
                M = big.tile([P, 2, N, K], f32, tag="Mt")
                nc.vector.tensor_tensor(out=M[:], in0=a_b, in1=w_b, op=MUL)
            else:
                w_b = w2[:, None, :, :].to_broadcast([P, K, 2, N])
                a_b = A[:, :, None, :].to_broadcast([P, K, 2, N])
                M = big.tile([P, K, 2, N], f32, tag="M")
                nc.vector.tensor_tensor(out=M[:], in0=a_b, in1=w_b, op=MUL)

            if t + 1 < T:
                gather(t + 1)

            if gp:
                # gpsimd add-tree over n: 100 -> 50 -> 25 -> 12(+1) -> 6 -> 3 -> 1
                s50 = big.tile([P, 2, 50, K], f32, tag="s50")
                nc.gpsimd.tensor_tensor(out=s50[:], in0=M[:, :, 0:50, :], in1=M[:, :, 50:100, :], op=ADD)
                s25 = big.tile([P, 2, 25, K], f32, tag="s25")
                nc.gpsimd.tensor_tensor(out=s25[:], in0=s50[:, :, 0:25, :], in1=s50[:, :, 25:50, :], op=ADD)
                s12 = big.tile([P, 2, 12, K], f32, tag="s12")
                nc.gpsimd.tensor_tensor(out=s12[:], in0=s25[:, :, 0:12, :], in1=s25[:, :, 12:24, :], op=ADD)
                s6 = big.tile([P, 2, 6, K], f32, tag="s6")
                nc.gpsimd.tensor_tensor(out=s6[:], in0=s12[:, :, 0:6, :], in1=s12[:, :, 6:12, :], op=ADD)
                s3 = big.tile([P, 2, 3, K], f32, tag="s3")
                nc.gpsimd.tensor_tensor(out=s3[:], in0=s6[:, :, 0:3, :], in1=s6[:, :, 3:6, :], op=ADD)
                t1 = big.tile([P, 2, 1, K], f32, tag="t1")
                nc.gpsimd.tensor_tensor(out=t1[:], in0=s3[:, :, 0:1, :], in1=s3[:, :, 1:2, :], op=ADD)
                t2 = big.tile([P, 2, 1, K], f32, tag="t2")
                nc.gpsimd.tensor_tensor(out=t2[:], in0=t1[:], in1=s3[:, :, 2:3, :], op=ADD)
                r2t = big.tile([P, 2, 1, K], f32, tag="r2t")
                nc.gpsimd.tensor_tensor(out=r2t[:], in0=t2[:], in1=s25[:, :, 24:25, :], op=ADD)
                nc.vector.tensor_scalar_add(probs[:, t, :], r2t[:, 0, 0, :], ft[:, 3 * N:3 * N + 1])
                nc.vector.tensor_scalar_add(negf[:, t, :], r2t[:, 1, 0, :], ft[:, 3 * N + 1:3 * N + 2])
            else:
                r2 = big.tile([P, K, 2], f32, tag="r2")
                nc.vector.tensor_reduce(out=r2[:], in_=M[:], axis=X, op=ADD)
                nc.vector.tensor_scalar_add(probs[:, t, :], r2[:, :, 0], ft[:, 3 * N:3 * N + 1])
                nc.vector.tensor_scalar_add(negf[:, t, :], r2[:, :, 1], ft[:, 3 * N + 1:3 * N + 2])

        negi = pool.tile([P, T, K], i32)
        nc.vector.tensor_copy(negi[:], negf[:])
        nc.sync.dma_start(oprob.rearrange("(t p) k -> p t k", p=P), probs[:])
        nc.sync.dma_start(oneg.rearrange("(t p) k -> p t k", p=P), negi[:])
    nc.compile()
    _cache["nc"] = nc
    return nc


def _prep_tables(knn_results, probs_table, cum_probs_table):
    c64 = np.asarray(cum_probs_table, dtype=np.float64)
    knn = np.asarray(knn_results)
    R = NLOC + 1

    ft = np.zeros((R, FTW), dtype=np.float32)
    ft[:, 0:N] = c64.astype(np.float32)
    # w_p[n] = (c[n+1]-c[n]) - (c[n]-c[n-1]), c[-1]=0, n<=98
    dc = c64[:, 1:N] - c64[:, 0:N - 1]                      # [R, 99] = c[n+1]-c[n], n=0..98
    pp = np.empty((R, N - 1), dtype=np.float64)             # p'[n] = c[n]-c[n-1], n=0..98
    pp[:, 0] = c64[:, 0]
    pp[:, 1:] = c64[:, 1:N - 1] - c64[:, 0:N - 2]
    ft[:, N:N + N - 1] = (dc - pp).astype(np.float32)
    # w_n[n] = nb[n+1]-nb[n], n<=98 (row loc holds knn[loc-1])
    nb64 = np.zeros((R, N), dtype=np.float64)
    nb64[1:] = knn.astype(np.float64)
    ft[:, 2 * N:2 * N + N - 1] = (nb64[:, 1:] - nb64[:, :-1]).astype(np.float32)
    ft[:, 3 * N] = c64[:, 0].astype(np.float32)             # c0
    ft[:, 3 * N + 1] = nb64[:, 0].astype(np.float32)        # nb0
    return np.ascontiguousarray(ft)


def make_in_maps(trg_seq, uniforms, knn_results, probs_table, cum_probs_table):
    trg_seq = np.ascontiguousarray(np.asarray(trg_seq, dtype=np.int32))
    uniforms = np.ascontiguousarray(np.asarray(uniforms, dtype=np.float32))
    ft = _prep_tables(knn_results, probs_table, cum_probs_table)
    in_maps = []
    for c in range(NCORES):
        sl = slice(c * RPC, (c + 1) * RPC)
        in_maps.append({
            "trg": trg_seq[sl],
            "uni": uniforms[sl],
            "ftab": ft,
        })
    return in_maps


def kernel(trg_seq, k, user, uniforms, knn_results, probs_table, cum_probs_table,
           **_ignored):
    nc = _build()
    in_maps = make_in_maps(trg_seq, uniforms, knn_results, probs_table, cum_probs_table)
    res = run_bass_kernel_spmd(nc, in_maps, core_ids=list(range(NCORES)))
    neg = np.concatenate([res.results[c]["oneg"] for c in range(NCORES)], axis=0)
    prob = np.concatenate([res.results[c]["oprob"] for c in range(NCORES)], axis=0)
    return neg, prob


# revision 12
# speedup vs baseline: 1.4895x; 1.4895x over previous
"""KNN WRMF negative sampler on 8 Trainium2 NeuronCores.

Data-parallel over L=4096 rows (512 rows/core, 4 tiles of 128 partitions).

Per row l with loc = trg_seq[l,1], for each of K=32 uniforms u:
  idx    = min(#{n: cum[loc,n] < u}, 99)
  neg    = knn[loc-1, idx]
  prob   = probs[loc, idx]        (computed as cum[idx]-cum[idx-1])

TRN2 has no per-lane gather, so both take-alongs are evaluated as
telescoped indicator sums over the prefix mask A[n] = 1[cum[n] < u]:

  neg  = nb[0] + sum_n w_n[n]*A[n],  w_n[n] = nb[n+1]-nb[n]          (n<=98)
  prob = c[0]  + sum_n w_p[n]*A[n],  w_p[n] = (c[n+1]-c[n])-(c[n]-c[n-1])

The clamp at 99 falls out of truncating the sums at n=98 (w[99]=0).
Weights, c0 and nb0 are precomputed on the host into a fused 2KB-row
table so each tile needs one [P,1]-offset indirect DMA (the only
indirect gather shape TRN2 SWDGE supports). neg arithmetic is exact
(integer-valued f32 partial sums < 2^24); prob error ~1e-5 relative.

Engines: DVE runs the three big passes back-to-back per tile (GT,
combined product, combined reduce); gpsimd only issues the row
gathers; ACT does the epilogue adds/cast off the DVE critical path.
"""

import numpy as np
from contextlib import ExitStack

import concourse.bass as bass
import concourse.bacc as bacc
import concourse.mybir as mybir
import concourse.tile as tile
from concourse.bass_utils import run_bass_kernel_spmd

P = 128          # partitions
T = 4            # row-tiles per core
RPC = P * T      # rows per core
K = 32           # samples per row
N = 100          # neighbours per row
FTW = 512        # fused table row width (f32 elems, 2KB)
NCORES = 8
NLOC = 100000

_cache = {}


def _build():
    if "nc" in _cache:
        return _cache["nc"]
    nc = bacc.Bacc("TRN2")
    f32, i32, bf16 = mybir.dt.float32, mybir.dt.int32, mybir.dt.bfloat16
    trg = nc.dram_tensor("trg", [RPC, 2], i32, kind="ExternalInput").ap()
    uni = nc.dram_tensor("uni", [RPC, K], f32, kind="ExternalInput").ap()
    ftab = nc.dram_tensor("ftab", [NLOC + 1, FTW], f32, kind="ExternalInput").ap()
    oneg = nc.dram_tensor("oneg", [RPC, K], i32, kind="ExternalOutput").ap()
    oprob = nc.dram_tensor("oprob", [RPC, K], f32, kind="ExternalOutput").ap()

    GT = mybir.AluOpType.is_gt
    ADD = mybir.AluOpType.add
    MUL = mybir.AluOpType.mult
    X = mybir.AxisListType.X

    with tile.TileContext(nc) as tc, ExitStack() as ctx:
        pool = ctx.enter_context(tc.tile_pool(name="m", bufs=1))
        big = ctx.enter_context(tc.tile_pool(name="big", bufs=2))
        ftp = ctx.enter_context(tc.tile_pool(name="ftp", bufs=4))

        # row -> (tile t, partition p): l = t*128 + p
        loc = pool.tile([P, T], i32)
        nc.sync.dma_start(loc[:], trg[:, 1:2].rearrange("(t p) c -> p (t c)", p=P))
        ut = pool.tile([P, T, K], f32)
        nc.sync.dma_start(ut[:], uni.rearrange("(t p) k -> p t k", p=P))

        probs = pool.tile([P, T, K], f32)
        negi = pool.tile([P, T, K], i32)
        oprob_r = oprob.rearrange("(t p) k -> p t k", p=P)
        oneg_r = oneg.rearrange("(t p) k -> p t k", p=P)

        fts = []

        def gather(t):
            ft = ftp.tile([P, FTW], f32, tag="ft")
            nc.gpsimd.indirect_dma_start(
                out=ft[:], out_offset=None, in_=ftab[:],
                in_offset=bass.IndirectOffsetOnAxis(ap=loc[:, t:t + 1], axis=0))
            fts.append(ft)

        gather(0)
        for t in range(T):
            ft = fts[t]
            u_b = ut[:, t, :][:, :, None].to_broadcast([P, K, N])
            c_b = ft[:, 0:N][:, None, :].to_broadcast([P, K, N])
            A = big.tile([P, K, N], bf16, tag="A")
            nc.vector.tensor_tensor(out=A[:], in0=u_b, in1=c_b, op=GT)

            # combined weighted products for both outputs: [P, K, 2, N]
            w_b = (ft[:, N:3 * N].rearrange("p (s n) -> p s n", s=2)
                   [:, None, :, :].to_broadcast([P, K, 2, N]))
            a_b = A[:, :, None, :].to_broadcast([P, K, 2, N])
            M = big.tile([P, K, 2, N], f32, tag="M")
            nc.vector.tensor_tensor(out=M[:], in0=a_b, in1=w_b, op=MUL)

            if t + 1 < T:
                gather(t + 1)

            r2 = big.tile([P, K, 2], f32, tag="r2")
            nc.vector.tensor_reduce(out=r2[:], in_=M[:], axis=X, op=ADD)

            # epilogue on the idle scalar engine: + c0 / + nb0, f32->i32
            nc.scalar.add(probs[:, t, :], r2[:, :, 0], ft[:, 3 * N:3 * N + 1])
            negf_t = big.tile([P, K], f32, tag="negf")
            nc.scalar.add(negf_t[:], r2[:, :, 1], ft[:, 3 * N + 1:3 * N + 2])
            nc.scalar.copy(negi[:, t, :], negf_t[:])

            if t % 2 == 1:
                hs = slice(t - 1, t + 1)
                nc.sync.dma_start(oprob_r[:, hs, :], probs[:, hs, :])
                nc.sync.dma_start(oneg_r[:, hs, :], negi[:, hs, :])
    nc.compile()
    _cache["nc"] = nc
    return nc


def _prep_tables(knn_results, probs_table, cum_probs_table):
    c64 = np.asarray(cum_probs_table, dtype=np.float64)
    knn = np.asarray(knn_results)
    R = NLOC + 1

    ft = np.zeros((R, FTW), dtype=np.float32)
    ft[:, 0:N] = c64.astype(np.float32)
    # w_p[n] = (c[n+1]-c[n]) - (c[n]-c[n-1]), c[-1]=0, n<=98
    dc = c64[:, 1:N] - c64[:, 0:N - 1]
    pp = np.empty((R, N - 1), dtype=np.float64)
    pp[:, 0] = c64[:, 0]
    pp[:, 1:] = c64[:, 1:N - 1] - c64[:, 0:N - 2]
    ft[:, N:N + N - 1] = (dc - pp).astype(np.float32)
    # w_n[n] = nb[n+1]-nb[n], n<=98 (row loc holds knn[loc-1])
    nb64 = np.zeros((R, N), dtype=np.float64)
    nb64[1:] = knn.astype(np.float64)
    ft[:, 2 * N:2 * N + N - 1] = (nb64[:, 1:] - nb64[:, :-1]).astype(np.float32)
    ft[:, 3 * N] = c64[:, 0].astype(np.float32)             # c0
    ft[:, 3 * N + 1] = nb64[:, 0].astype(np.float32)        # nb0
    return np.ascontiguousarray(ft)


def make_in_maps(trg_seq, uniforms, knn_results, probs_table, cum_probs_table):
    trg_seq = np.ascontiguousarray(np.asarray(trg_seq, dtype=np.int32))
    uniforms = np.ascontiguousarray(np.asarray(uniforms, dtype=np.float32))
    ft = _prep_tables(knn_results, probs_table, cum_probs_table)
    in_maps = []
    for c in range(NCORES):
        sl = slice(c * RPC, (c + 1) * RPC)
        in_maps.append({
            "trg": trg_seq[sl],
            "uni": uniforms[sl],
            "ftab": ft,
        })
    return in_maps


def kernel(trg_seq, k, user, uniforms, knn_results, probs_table, cum_probs_table,
           **_ignored):
    nc = _build()
    in_maps = make_in_maps(trg_seq, uniforms, knn_results, probs_table, cum_probs_table)
    res = run_bass_kernel_spmd(nc, in_maps, core_ids=list(range(NCORES)))
    neg = np.concatenate([res.results[c]["oneg"] for c in range(NCORES)], axis=0)
    prob = np.concatenate([res.results[c]["oprob"] for c in range(NCORES)], axis=0)
    return neg, prob


# revision 13
# speedup vs baseline: 1.5188x; 1.0197x over previous
"""KNN WRMF negative sampler on 8 Trainium2 NeuronCores.

Data-parallel over L=4096 rows (512 rows/core, 4 tiles of 128 partitions).

Per row l with loc = trg_seq[l,1], for each of K=32 uniforms u:
  idx    = min(#{n: cum[loc,n] < u}, 99)
  neg    = knn[loc-1, idx]
  prob   = probs[loc, idx]        (computed as cum[idx]-cum[idx-1])

TRN2 has no per-lane gather, so both take-alongs are evaluated as
telescoped indicator sums over the prefix mask A[n] = 1[cum[n] < u]:

  neg  = nb[0] + sum_n w_n[n]*A[n],  w_n[n] = nb[n+1]-nb[n]          (n<=98)
  prob = c[0]  + sum_n w_p[n]*A[n],  w_p[n] = (c[n+1]-c[n])-(c[n]-c[n-1])

The clamp at 99 falls out of truncating the sums at n=98 (w[99]=0).
Weights, c0 and nb0 are precomputed on the host into a fused 2KB-row
table so each tile needs one [P,1]-offset indirect DMA (the only
indirect gather shape TRN2 SWDGE supports). neg arithmetic is exact
(integer-valued f32 partial sums < 2^24); prob error ~1e-5 relative.

Engines: DVE runs the three big passes back-to-back per tile (GT,
combined product, combined reduce); gpsimd only issues the row
gathers; ACT does the epilogue adds/cast off the DVE critical path.
"""

import numpy as np
from contextlib import ExitStack

import concourse.bass as bass
import concourse.bacc as bacc
import concourse.mybir as mybir
import concourse.tile as tile
from concourse.bass_utils import run_bass_kernel_spmd

P = 128          # partitions
T = 4            # row-tiles per core
RPC = P * T      # rows per core
K = 32           # samples per row
N = 100          # neighbours per row
FTW = 512        # fused table row width (f32 elems, 2KB)
NCORES = 8
NLOC = 100000

_cache = {}


def _build():
    if "nc" in _cache:
        return _cache["nc"]
    nc = bacc.Bacc("TRN2")
    f32, i32, f16 = mybir.dt.float32, mybir.dt.int32, mybir.dt.float16
    trg = nc.dram_tensor("trg", [RPC, 2], i32, kind="ExternalInput").ap()
    uni = nc.dram_tensor("uni", [RPC, K], f32, kind="ExternalInput").ap()
    ftab = nc.dram_tensor("ftab", [NLOC + 1, FTW], f32, kind="ExternalInput").ap()
    oneg = nc.dram_tensor("oneg", [RPC, K], i32, kind="ExternalOutput").ap()
    oprob = nc.dram_tensor("oprob", [RPC, K], f32, kind="ExternalOutput").ap()

    GT = mybir.AluOpType.is_gt
    ADD = mybir.AluOpType.add
    MUL = mybir.AluOpType.mult
    X = mybir.AxisListType.X

    with tile.TileContext(nc) as tc, ExitStack() as ctx:
        pool = ctx.enter_context(tc.tile_pool(name="m", bufs=1))
        big = ctx.enter_context(tc.tile_pool(name="big", bufs=2))
        ftp = ctx.enter_context(tc.tile_pool(name="ftp", bufs=4))

        # row -> (tile t, partition p): l = t*128 + p
        loc = pool.tile([P, T], i32)
        nc.sync.dma_start(loc[:], trg[:, 1:2].rearrange("(t p) c -> p (t c)", p=P))
        ut = pool.tile([P, T, K], f32)
        nc.sync.dma_start(ut[:], uni.rearrange("(t p) k -> p t k", p=P))

        probs = pool.tile([P, T, K], f32)
        negi = pool.tile([P, T, K], i32)
        oprob_r = oprob.rearrange("(t p) k -> p t k", p=P)
        oneg_r = oneg.rearrange("(t p) k -> p t k", p=P)

        fts = []

        def gather(t):
            ft = ftp.tile([P, FTW], f32, tag="ft")
            nc.gpsimd.indirect_dma_start(
                out=ft[:], out_offset=None, in_=ftab[:],
                in_offset=bass.IndirectOffsetOnAxis(ap=loc[:, t:t + 1], axis=0))
            fts.append(ft)

        gather(0)
        for t in range(T):
            ft = fts[t]
            u_b = ut[:, t, :][:, :, None].to_broadcast([P, K, N])
            c_b = ft[:, 0:N][:, None, :].to_broadcast([P, K, N])
            A = big.tile([P, K, N], f16, tag="A")
            nc.vector.tensor_tensor(out=A[:], in0=u_b, in1=c_b, op=GT)

            # f16 weighted products for 3 streams (prob, knn-hi, knn-lo):
            # all-f16 packed-last-dim operands run the DVE 2x mode
            w16 = (ft[:, N:N + 3 * N // 2].bitcast(f16)
                   .rearrange("p (s n) -> p s n", s=3))
            w_b = w16[:, None, :, :].to_broadcast([P, K, 3, N])
            a_b = A[:, :, None, :].to_broadcast([P, K, 3, N])
            M = big.tile([P, K, 3, N], f16, tag="M")
            nc.vector.tensor_tensor(out=M[:], in0=a_b, in1=w_b, op=MUL)

            if t + 1 < T:
                gather(t + 1)

            # prob: single-pass reduce with f32 accumulation
            rp = big.tile([P, K], f32, tag="rp")
            nc.vector.tensor_reduce(out=rp[:], in_=M[:, :, 0, :], axis=X, op=ADD)

            # knn hi/lo: exact f16 add-tree (all node sums are range
            # differences of hi<=97 / lo<=1023, exactly representable)
            Mk = M[:, :, 1:3, :]
            s50 = big.tile([P, K, 2, 50], f16, tag="s50")
            nc.vector.tensor_tensor(out=s50[:], in0=Mk[:, :, :, 0:50], in1=Mk[:, :, :, 50:100], op=ADD)
            s25 = big.tile([P, K, 2, 25], f16, tag="s25")
            nc.vector.tensor_tensor(out=s25[:], in0=s50[:, :, :, 0:25], in1=s50[:, :, :, 25:50], op=ADD)
            s12 = big.tile([P, K, 2, 12], f16, tag="s12")
            nc.vector.tensor_tensor(out=s12[:], in0=s25[:, :, :, 0:12], in1=s25[:, :, :, 12:24], op=ADD)
            s6 = big.tile([P, K, 2, 6], f16, tag="s6")
            nc.vector.tensor_tensor(out=s6[:], in0=s12[:, :, :, 0:6], in1=s12[:, :, :, 6:12], op=ADD)
            s3 = big.tile([P, K, 2, 3], f16, tag="s3")
            nc.vector.tensor_tensor(out=s3[:], in0=s6[:, :, :, 0:3], in1=s6[:, :, :, 3:6], op=ADD)
            t1 = big.tile([P, K, 2, 1], f16, tag="t1")
            nc.vector.tensor_tensor(out=t1[:], in0=s3[:, :, :, 0:1], in1=s3[:, :, :, 1:2], op=ADD)
            t2 = big.tile([P, K, 2, 1], f16, tag="t2")
            nc.vector.tensor_tensor(out=t2[:], in0=t1[:], in1=s3[:, :, :, 2:3], op=ADD)
            rk = big.tile([P, K, 2, 1], f16, tag="rk")
            nc.vector.tensor_tensor(out=rk[:], in0=t2[:], in1=s25[:, :, :, 24:25], op=ADD)

            # neg = nb0 + 1024*hi + lo ; prob = c0 + rp  (small ops)
            negh = big.tile([P, K], f32, tag="negh")
            nc.vector.tensor_scalar_mul(negh[:], rk[:, :, 0, 0], 1024)
            negf_t = big.tile([P, K], f32, tag="negf")
            nc.vector.tensor_tensor(out=negf_t[:], in0=negh[:], in1=rk[:, :, 1, 0], op=ADD)

            nc.scalar.add(probs[:, t, :], rp[:], ft[:, 3 * N:3 * N + 1])
            negf2 = big.tile([P, K], f32, tag="negf2")
            nc.scalar.add(negf2[:], negf_t[:], ft[:, 3 * N + 1:3 * N + 2])
            nc.scalar.copy(negi[:, t, :], negf2[:])

            if t % 2 == 1:
                hs = slice(t - 1, t + 1)
                nc.sync.dma_start(oprob_r[:, hs, :], probs[:, hs, :])
                nc.sync.dma_start(oneg_r[:, hs, :], negi[:, hs, :])
    nc.compile()
    _cache["nc"] = nc
    return nc


def _prep_tables(knn_results, probs_table, cum_probs_table):
    c64 = np.asarray(cum_probs_table, dtype=np.float64)
    knn = np.asarray(knn_results)
    R = NLOC + 1

    ft = np.zeros((R, FTW), dtype=np.float32)
    ft[:, 0:N] = c64.astype(np.float32)
    # f16 weight blocks at f32 slots [N : N + 3N/2): w_p | w_hi | w_lo
    dc = c64[:, 1:N] - c64[:, 0:N - 1]
    pp = np.empty((R, N - 1), dtype=np.float64)
    pp[:, 0] = c64[:, 0]
    pp[:, 1:] = c64[:, 1:N - 1] - c64[:, 0:N - 2]
    nb64 = np.zeros((R, N), dtype=np.int64)
    nb64[1:] = knn
    hi64 = nb64 >> 10
    lo64 = nb64 & 1023
    w16 = np.zeros((R, 3, N), dtype=np.float16)
    w16[:, 0, :N - 1] = (dc - pp).astype(np.float16)
    w16[:, 1, :N - 1] = (hi64[:, 1:] - hi64[:, :-1]).astype(np.float16)
    w16[:, 2, :N - 1] = (lo64[:, 1:] - lo64[:, :-1]).astype(np.float16)
    ft[:, N:N + 3 * N // 2].view(np.float16)[:] = w16.reshape(R, 3 * N)
    ft[:, 3 * N] = c64[:, 0].astype(np.float32)             # c0
    ft[:, 3 * N + 1] = nb64[:, 0].astype(np.float32)        # nb0 (=0 pad row ok)
    return np.ascontiguousarray(ft)


def make_in_maps(trg_seq, uniforms, knn_results, probs_table, cum_probs_table):
    trg_seq = np.ascontiguousarray(np.asarray(trg_seq, dtype=np.int32))
    uniforms = np.ascontiguousarray(np.asarray(uniforms, dtype=np.float32))
    ft = _prep_tables(knn_results, probs_table, cum_probs_table)
    in_maps = []
    for c in range(NCORES):
        sl = slice(c * RPC, (c + 1) * RPC)
        in_maps.append({
            "trg": trg_seq[sl],
            "uni": uniforms[sl],
            "ftab": ft,
        })
    return in_maps


def kernel(trg_seq, k, user, uniforms, knn_results, probs_table, cum_probs_table,
           **_ignored):
    nc = _build()
    in_maps = make_in_maps(trg_seq, uniforms, knn_results, probs_table, cum_probs_table)
    res = run_bass_kernel_spmd(nc, in_maps, core_ids=list(range(NCORES)))
    neg = np.concatenate([res.results[c]["oneg"] for c in range(NCORES)], axis=0)
    prob = np.concatenate([res.results[c]["oprob"] for c in range(NCORES)], axis=0)
    return neg, prob
